# revision 1
# baseline (speedup 1.0000x reference)
"""GRU decoder kernel for Trainium2 (8 NeuronCores, data-parallel over batch).

Problem (hardcoded): B=4096, T=168, D=64, H=128.
  per step t:  gx_t = feats_t @ W_ih[:, :D].T + b_ih + y_prev * w_y
               gh   = h @ W_hh.T + b_hh
               r = sig(gx_r+gh_r); z = sig(gx_z+gh_z)
               n = tanh(gx_n + r*gh_n)
               h = (1-z)*n + z*h;  y = h @ wo + bo

Mapping per core: batch shard BS=512 split into TWO interleaved chains of
C=256 columns each, so the serial per-step latency of one chain hides
behind the other chain's engine work (keeps PE dense -> HAM stays warm).

Per chain-step, layout [hidden dim on partitions, batch on free]:
  - feats arrive HOST-PRE-TRANSPOSED as [d, b] tiles (rows 0:64 = even t,
    64:128 = odd t) -> no PE transposes / evac copies on device.
  - p_rz psum bank [128, 512] holds r|z gates side by side: accumulated
    from feats matmuls (K=64, even/odd row-packed), a K=2 ones-matmul
    adding the per-gate biases, and W1 @ h (W1 = W_hh + w_y (x) wo folds
    the y-feedback for t>=1).  ONE fused ACT sigmoid (FD=512) -> r16|z16.
  - p_gxh psum bank: gx_n (feats + wynt@h rank-1 y-feedback) | gh_n.
    t1 = (gh_n + bhn) * r   (DVE STT),  npre = (gx_n + bn) + t1 (DVE STT),
    n = tanh(npre) (ACT).
  - h' = (1-z)*n + z*h via zc=1-z (gpsimd), zh=z*h (gpsimd, off critical
    path), zn=zc*n (DVE), h'=zn+zh (DVE).
  - y_t = wo . h_t via M=32 matmul into psum col-group t%4; every 4 steps
    one ACT copy evacuates to SBUF and a small DMA writes yT[4g:4g+4].
    bo is added on the host after gather.
  - Step 0 uses the supplied y0 via K=4 (bias+y0 for r/z) and K=1 (n gate)
    matmuls with rhs patterns from the pack tensor.
Emission is phase-ordered across the two chains so each engine's FIFO
matches the pipelined timeline (sigA, sigB, tanhA, tanhB on ACT; ...).
"""

import numpy as np

import concourse.bacc as bacc
import concourse.bass as bass
import concourse.mybir as mybir
import concourse.tile as tile
from concourse.bass_utils import run_bass_kernel_spmd

B, T, D, H = 4096, 168, 64, 128
NCORES = 8
BS = B // NCORES  # 512
C = BS // 2       # 256 per chain

F32 = mybir.dt.float32
F16 = mybir.dt.float16
AF = mybir.ActivationFunctionType
ALU = mybir.AluOpType

CH = 12  # feats t-pairs per DMA chunk

# pack (fp16 [128, NPACK]) column layout
_WOC0 = 0      # [128, 32]  wo duplicated 32x
_B0L = 160     # rows 0:2, 128 cols: wy_r, wy_z (t=0 y0-feedback lhsT)
_WYN = 288     # row 0, 128 cols: wy_n
_RHS0 = 928    # rows 0:2, 512 cols per chain (x2): y0|0, 0|y0
_Y0R = 1952    # row 0, 256 cols per chain (x2): y0
_FB = 2464     # [128, 16] fp16 = [128, 8] fp32 bitcast:
               #   bhn, bn1, bn0, bo, brz1_r, brz1_z, brz0_r, brz0_z
_ID0 = 2480    # [128, 128] identity (for t1 -> gx psum accumulate)
NPACK = 2608


def build(nt=T):
    """Build the per-core Bass program. nt: number of timesteps (tests)."""
    assert nt % 4 == 0
    npairs = nt // 2
    ch = min(CH, npairs)
    nchunk = (npairs + ch - 1) // ch
    nc = bacc.Bacc("TRN2", target_bir_lowering=False, debug=False)

    featsT = nc.declare_dram_parameter("featsT", [256, npairs, C], F16, isOutput=False)
    h0T_d = nc.declare_dram_parameter("h0T", [128, BS], F16, isOutput=False)
    wft_d = nc.declare_dram_parameter("wft", [128, 384], F16, isOutput=False)
    w1t_d = nc.declare_dram_parameter("w1t", [128, 256], F16, isOutput=False)
    whhnt_d = nc.declare_dram_parameter("whhnt", [128, 128], F16, isOutput=False)
    wynt_d = nc.declare_dram_parameter("wynt", [128, 128], F16, isOutput=False)
    whhrz0_d = nc.declare_dram_parameter("whhrz0", [128, 256], F16, isOutput=False)
    pack = nc.declare_dram_parameter("pack", [128, NPACK], F16, isOutput=False)

    yT = nc.declare_dram_parameter("yT", [nt, BS], F16, isOutput=True)

    with tile.TileContext(nc) as tc:
        with (
            tc.tile_pool(name="wpool", bufs=1) as wpool,
            tc.tile_pool(name="fpool", bufs=2) as fpool,
            tc.tile_pool(name="hpool", bufs=2) as hpool,
            tc.tile_pool(name="gpool", bufs=2) as gpool,
            tc.tile_pool(name="ypool", bufs=2) as ypool,
            tc.tile_pool(name="ps_rz", bufs=2, space="PSUM") as ps_rz,
            tc.tile_pool(name="ps_gxh", bufs=1, space="PSUM") as ps_gxh,
            tc.tile_pool(name="ps_u", bufs=1, space="PSUM") as ps_u,
        ):
            # ---- constants ----
            pk = wpool.tile([128, NPACK], F16)
            nc.sync.dma_start(pk[:], pack[:])
            woc = pk[:, _WOC0:_WOC0 + 32]
            wyrzl = pk[0:2, _B0L:_B0L + 128]
            wynr = pk[0:1, _WYN:_WYN + 128]
            rhs0 = [pk[0:2, _RHS0 + 512 * c2:_RHS0 + 512 * (c2 + 1)] for c2 in (0, 1)]
            y0r = [pk[0:1, _Y0R + 256 * c2:_Y0R + 256 * (c2 + 1)] for c2 in (0, 1)]
            fb = pk[:, _FB:_FB + 16].bitcast(F32)
            bhn, bn1, bn0 = fb[:, 0:1], fb[:, 1:2], fb[:, 2:3]
            brz1_r, brz1_z = fb[:, 4:5], fb[:, 5:6]
            brz0_r, brz0_z = fb[:, 6:7], fb[:, 7:8]
            ident = pk[:, _ID0:_ID0 + 128]

            wft = wpool.tile([128, 384], F16)
            w1t = wpool.tile([128, 256], F16)
            whhnt = wpool.tile([128, 128], F16)
            wynt = wpool.tile([128, 128], F16)
            whhrz0 = wpool.tile([128, 256], F16)
            for sb, dr in [
                (wft, wft_d), (w1t, w1t_d), (whhnt, whhnt_d),
                (wynt, wynt_d), (whhrz0, whhrz0_d),
            ]:
                nc.sync.dma_start(sb[:], dr[:])
            h0sb = wpool.tile([128, BS], F16)
            nc.sync.dma_start(h0sb[:], h0T_d[:])

            hprev = [h0sb[:, 0:C], h0sb[:, C:2 * C]]

            # ---- feats chunks ----
            fchunks = [{}, {}]

            def load_chunk(c2, ci):
                p0 = ci * ch
                pn = min(ch, npairs - p0)
                ft = fpool.tile([128, ch * C], F16, tag=f"ft{c2}")
                nc.sync.dma_start(
                    ft[:, :pn * C], featsT[c2 * 128:(c2 + 1) * 128, p0:p0 + pn, :]
                )
                fchunks[c2][ci] = ft

            for c2 in (0, 1):
                load_chunk(c2, 0)

            przs = [{}, {}]
            gxhs = [{}, {}]
            pus = [None, None]
            rzs = [None, None]
            t1s = [None, None]
            npres = [None, None]
            n16s = [None, None]
            zcs = [None, None]
            zhs = [None, None]

            def emit_pair(c2, t):
                # feats + bias matmuls for steps (t, t+1); even t rows 0:64,
                # odd t rows 64:128 run concurrently on disjoint PE rows.
                p = t // 2
                ci, po = divmod(p, ch)
                ft = fchunks[c2][ci]
                for tt, half in ((t, 0), (t + 1, 64)):
                    if tt >= nt:
                        break
                    fh = ft[half:half + 64, po * C:(po + 1) * C]
                    w = wft[half:half + 64, :]
                    tp = (half, 0)
                    prz = ps_rz.tile([128, 2 * C], F32, tag=f"rz{c2}")
                    gxh = ps_gxh.tile([128, C], F32, tag=f"gxh{c2}")
                    # feats-r opens the rz bank group
                    nc.tensor.matmul(prz[:, 0:C], w[:, 0:128], fh,
                                     start=True, stop=False, tile_position=tp)
                    nc.tensor.matmul(prz[:, C:2 * C], w[:, 128:256], fh,
                                     start=False, stop=False, tile_position=tp)
                    if tt == 0:
                        # y0 feedback for r/z: K=2 (wy_r, wy_z) x (y0|0, 0|y0)
                        nc.tensor.matmul(prz[:], wyrzl, rhs0[c2],
                                         start=False, stop=False)
                    # feats-n opens the gx bank group
                    nc.tensor.matmul(gxh[:], w[:, 256:384], fh,
                                     start=True, stop=False, tile_position=tp)
                    if tt == 0:
                        nc.tensor.matmul(gxh[:], wynr, y0r[c2],
                                         start=False, stop=False)
                    przs[c2][tt] = prz
                    gxhs[c2][tt] = gxh

            def emit_hidden_sig(c2, t):
                hp = hprev[c2]
                prz = przs[c2][t]
                gxh = gxhs[c2][t]
                wrz = whhrz0 if t == 0 else w1t
                nc.tensor.matmul(prz[:, 0:C], wrz[:, 0:128], hp,
                                 start=False, stop=False)
                nc.tensor.matmul(prz[:, C:2 * C], wrz[:, 128:256], hp,
                                 start=False, stop=True)
                if t > 0:
                    nc.tensor.matmul(gxh[:], wynt[:], hp,
                                     start=False, stop=False)
                rz = gpool.tile([128, 2 * C], F16, tag=f"rz16{c2}")
                nc.scalar.activation(rz[:, 0:C], prz[:, 0:C], AF.Sigmoid,
                                     bias=brz0_r if t == 0 else brz1_r)
                nc.scalar.activation(rz[:, C:2 * C], prz[:, C:2 * C], AF.Sigmoid,
                                     bias=brz0_z if t == 0 else brz1_z)
                rzs[c2] = rz
                # gh = whhnt @ h is parked in the now-dead r half of the rz
                # bank (start+stop single-instruction group, overwrites after
                # sig_r's read)
                nc.tensor.matmul(prz[:, 0:C], whhnt[:], hp,
                                 start=True, stop=True)

            def emit_t1_npre(c2, t):
                prz = przs[c2][t]
                gxh = gxhs[c2][t]
                r16 = rzs[c2][:, 0:C]
                t1 = gpool.tile([128, C], F16, tag=f"t1{c2}")
                nc.vector.scalar_tensor_tensor(
                    t1[:], prz[:, 0:C], bhn, r16, ALU.add, ALU.mult)
                # accumulate t1 into the gx psum on the PE; tanh reads psum
                nc.tensor.matmul(gxh[:], ident, t1[:],
                                 start=False, stop=True)
                t1s[c2] = t1

            def emit_zc_zh(c2, t):
                z16 = rzs[c2][:, C:2 * C]
                zc = gpool.tile([128, C], F16, tag=f"zc{c2}")
                nc.gpsimd.tensor_scalar(zc[:], z16, -1.0, 1.0, ALU.mult, ALU.add)
                zh = gpool.tile([128, C], F16, tag=f"zh{c2}")
                nc.gpsimd.tensor_tensor(zh[:], z16, hprev[c2], ALU.mult)
                zcs[c2] = zc
                zhs[c2] = zh

            def emit_tanh(c2, t):
                gxh = gxhs[c2].pop(t)
                n16 = gpool.tile([128, C], F16, tag=f"n16{c2}")
                nc.scalar.activation(n16[:], gxh[:], AF.Tanh,
                                     bias=bn0 if t == 0 else bn1)
                n16s[c2] = n16

            def emit_combine_y(c2, t):
                przs[c2].pop(t)
                zn = gpool.tile([128, C], F16, tag=f"zn{c2}")
                nc.vector.tensor_tensor(zn[:], zcs[c2][:], n16s[c2][:], ALU.mult)
                hT = hpool.tile([128, C], F16, tag=f"h{c2}")
                nc.vector.tensor_tensor(hT[:], zn[:], zhs[c2][:], ALU.add)
                hprev[c2] = hT
                c4 = t % 4
                if c4 == 0:
                    pus[c2] = ps_u.tile([128, C], F32, tag=f"u{c2}", name=f"pu{c2}")
                nc.tensor.matmul(
                    pus[c2][32 * c4:32 * (c4 + 1), :], woc, hT[:],
                    start=True, stop=True, tile_position=(0, 32 * c4),
                )
                if c4 == 3:
                    g = t // 4
                    yf = ypool.tile([128, C], F16, tag=f"yf{c2}")
                    nc.vector.tensor_copy(yf[:], pus[c2][:])
                    nc.sync.dma_start(
                        yT[4 * g:4 * (g + 1), c2 * C:(c2 + 1) * C],
                        yf[0:128:32, :],
                    )

            for t in range(nt):
                if t % 2 == 0:
                    p = t // 2
                    ci = p // ch
                    for c2 in (0, 1):
                        if p % ch == 0 and ci + 1 < nchunk:
                            load_chunk(c2, ci + 1)
                        emit_pair(c2, t)
                for c2 in (0, 1):
                    emit_hidden_sig(c2, t)
                for c2 in (0, 1):
                    emit_t1_npre(c2, t)
                for c2 in (0, 1):
                    emit_zc_zh(c2, t)
                for c2 in (0, 1):
                    emit_tanh(c2, t)
                for c2 in (0, 1):
                    emit_combine_y(c2, t)

    nc.compile()
    return nc


# -------- host-side weight prep + sharded execution --------

def _prep_aux(W_ih, W_hh, b_ih, b_hh, Wo, bo):
    W_ih = np.asarray(W_ih, np.float32)
    W_hh = np.asarray(W_hh, np.float32)
    b_ih = np.asarray(b_ih, np.float32)
    b_hh = np.asarray(b_hh, np.float32)
    wo = np.asarray(Wo, np.float32)[0]       # [H]
    bo_s = float(np.asarray(bo, np.float32)[0])
    wfd = W_ih[:, :D]                         # [3H, D]
    w_y = W_ih[:, D]                          # [3H]

    wft = np.zeros((128, 384), np.float16)
    wft[0:64] = wfd.T.astype(np.float16)
    wft[64:128] = wfd.T.astype(np.float16)

    W1 = W_hh[0:2 * H] + np.outer(w_y[0:2 * H], wo)       # [2H, H]
    aux = dict(
        wft=wft,
        w1t=np.ascontiguousarray(W1.T.astype(np.float16)),
        whhnt=np.ascontiguousarray(W_hh[2 * H:].T.astype(np.float16)),
        wynt=np.ascontiguousarray(np.outer(wo, w_y[2 * H:]).astype(np.float16)),
        whhrz0=np.ascontiguousarray(W_hh[0:2 * H].T.astype(np.float16)),
    )

    pk = np.zeros((128, NPACK), np.float16)
    pk[:, _WOC0:_WOC0 + 32] = np.repeat(wo[:, None], 32, axis=1).astype(np.float16)
    brz_base = (b_ih + b_hh)[0:2 * H]
    pk[0, _B0L:_B0L + 128] = w_y[0:H].astype(np.float16)
    pk[1, _B0L:_B0L + 128] = w_y[H:2 * H].astype(np.float16)
    pk[0, _WYN:_WYN + 128] = w_y[2 * H:].astype(np.float16)
    fbv = np.stack(
        [b_hh[2 * H:],                          # bhn
         b_ih[2 * H:] + w_y[2 * H:] * bo_s,     # bn1
         b_ih[2 * H:],                          # bn0
         np.full(128, bo_s, np.float32),        # bo (unused on device)
         brz_base[0:H] + w_y[0:H] * bo_s,       # brz1_r
         brz_base[H:2 * H] + w_y[H:2 * H] * bo_s,  # brz1_z
         brz_base[0:H],                         # brz0_r
         brz_base[H:2 * H]],                    # brz0_z
        axis=1,
    ).astype(np.float32)
    pk[:, _FB:_FB + 16] = np.ascontiguousarray(fbv).view(np.float16)
    pk[:, _ID0:_ID0 + 128] = np.eye(128, dtype=np.float16)
    aux["pack"] = pk
    aux["bo_s"] = bo_s
    return aux


def _core_featsT(ff_core):
    """[BS, nt, D] fp16 -> [256, nt//2, C]: rows = c2*128 + (t%2)*64 + d."""
    nt = ff_core.shape[1]
    a = ff_core.reshape(2, C, nt // 2, 2, D).transpose(0, 3, 4, 2, 1)
    return np.ascontiguousarray(a).reshape(256, nt // 2, C)


def _fill_y0(pkc, y0c):
    """Write per-core y0 (fp16 [BS]) into the pack's rhs0 / y0r slots."""
    for c2 in (0, 1):
        sl = y0c[c2 * C:(c2 + 1) * C]
        pkc[0, _RHS0 + 512 * c2:_RHS0 + 512 * c2 + 256] = sl
        pkc[1, _RHS0 + 512 * c2 + 256:_RHS0 + 512 * (c2 + 1)] = sl
        pkc[0, _Y0R + 256 * c2:_Y0R + 256 * (c2 + 1)] = sl


_NC_CACHE = {}


def kernel(future_feats, h0, y0, W_ih, W_hh, b_ih, b_hh, Wo, bo):
    ff = np.asarray(future_feats).astype(np.float16)      # [B, T, D]
    h0f = np.asarray(h0).astype(np.float16)[0]            # [B, H]
    y0f = np.asarray(y0).astype(np.float16)               # [B]

    aux = _prep_aux(W_ih, W_hh, b_ih, b_hh, Wo, bo)
    bo_s = aux.pop("bo_s")

    if "nc" not in _NC_CACHE:
        _NC_CACHE["nc"] = build(T)
    nc = _NC_CACHE["nc"]

    in_maps = []
    for c in range(NCORES):
        sl = slice(c * BS, (c + 1) * BS)
        m = dict(aux)
        pkc = aux["pack"].copy()
        _fill_y0(pkc, y0f[sl])
        m["pack"] = pkc
        m["featsT"] = _core_featsT(ff[sl])
        m["h0T"] = np.ascontiguousarray(h0f[sl].T)
        in_maps.append(m)

    res = run_bass_kernel_spmd(nc, in_maps, core_ids=list(range(NCORES)))
    outs = [r["yT"] for r in res.results]
    out = np.concatenate([o.T.astype(np.float32) for o in outs], axis=0)
    return out + bo_s

